# revision 11
# baseline (speedup 1.0000x reference)
"""Trainium2 Bass kernel for a 2-layer bidirectional SRU text classifier.

Model (see reference):
    e  = embed[x]                              [T, B, D]
    h0 = BiSRU(e;  W0f/b0f, W0b/b0b)           [T, B, 2H]
    h1 = BiSRU(h0; W1f/b1f, W1b/b1b)           [T, B, 2H]
    out = tanh(max_t tanh(h1)) @ Wh + bh       [B, C]

T=512, B=64, V=50000, D=300, H=512, C=10.

Strategy: data-parallel over batch across 8 NeuronCores (8 sequences per
core), weights/embedding replicated.  On each core everything is kept in a
[feature, (b, t)] layout so the SRU recurrence runs as a hardware
``tensor_tensor_scan`` along the free (time) axis, and the matmuls
contract over features on the partition axis.  The backward direction is
computed in reversed-time coordinates: the embedding transpose uses an
anti-diagonal identity so the input arrives time-reversed, and the layer-0
output is written back with a negative-stride access pattern.

Two phases per core (to fit SBUF):
  E : embedding gather (indirect DMA) + PE transpose -> e^T; layer-0
      forward+backward SRU; spill h0 to DRAM.
  L1: reload h0 per sequence, layer-1 forward pass then backward pass
      (separate weight residency), max-pool + final classifier.
"""

import numpy as np

T, B, V, D, H, C = 512, 64, 50000, 300, 512, 10
NCORES = 8
BL = B // NCORES  # sequences per core

# matmul operand dtype: "float32r" (full fp32 storage, fast PE mode),
# "bfloat16", or "float32" (4x slower PE).
MM_DTYPE = "float32r"
# If True, feed reversed access patterns directly to matmul rhs instead of
# materializing reversed copies of h0 (saves DVE work + SBUF).
REV_RHS = False

KCH0 = [(0, 128), (1, 128), (2, 44)]  # layer-0 K chunks over D=300
NK1 = 8  # layer-1 K chunks over 2H=1024


def build_program(mm_dtype=MM_DTYPE, rev_rhs=REV_RHS):
    import concourse.bacc as bacc
    import concourse.bass as bass  # noqa: F401
    import concourse.mybir as mybir
    import concourse.tile as tile
    from concourse.bass import IndirectOffsetOnAxis
    from concourse.masks import make_identity

    dt = mybir.dt
    f32 = dt.float32
    i32 = dt.int32
    Alu = mybir.AluOpType
    Act = mybir.ActivationFunctionType

    DTM = getattr(dt, mm_dtype)  # matmul operand dtype
    # Storage dtype of matmul operand tiles.  For float32r the tiles are
    # typed float32r end-to-end (the BIR verifier requires producers of
    # fp32r-matmul operands to emit fp32r), and fp32 DRAM sources are
    # bitcast on load (same 4-byte layout).
    DTS = DTM

    def mmv(ap):
        return ap

    def dview(dram_ap):
        """View an fp32 DRAM AP with the operand dtype for cast-free DMA."""
        if mm_dtype == "float32r":
            return dram_ap.bitcast(DTM)
        return dram_ap

    nc = bacc.Bacc()

    x_t = nc.declare_dram_parameter("x", [T, BL], i32, isOutput=False)
    emb_t = nc.declare_dram_parameter("embed", [V, D], f32, isOutput=False)
    w_t = {}
    b_t = {}
    for nm, shp in (("W0f", [D, 4 * H]), ("W0b", [D, 4 * H]),
                    ("W1f", [2 * H, 4 * H]), ("W1b", [2 * H, 4 * H])):
        w_t[nm] = nc.declare_dram_parameter(nm, shp, f32, isOutput=False)
    for nm in ("b0f", "b0b", "b1f", "b1b"):
        b_t[nm] = nc.declare_dram_parameter(nm, [2 * H], f32, isOutput=False)
    wh_t = nc.declare_dram_parameter("Wh", [2 * H, C], f32, isOutput=False)
    bh_t = nc.declare_dram_parameter("bh", [C], f32, isOutput=False)
    out_t = nc.declare_dram_parameter("out", [C, BL], f32, isOutput=True)

    h0_dram = nc.dram_tensor("h0_stage", [BL, 128, NK1, T], DTS)

    with tile.TileContext(nc) as tc:
        with tc.tile_pool(name="const", bufs=1) as constp, \
             tc.tile_pool(name="w1fp", bufs=1) as w1fp:
            # ---- constants ----
            ident = constp.tile([128, 128], f32, tag="ident")
            make_identity(nc, ident[:, :])
            antid = constp.tile([128, 128], f32, tag="antid")
            nc.gpsimd.memset(antid[:, :], 0.0)
            # out[x, y] = 1.0 where x + y - 127 == 0 (anti-diagonal)
            nc.gpsimd.affine_select(
                out=antid[:, :], in_=antid[:, :],
                compare_op=Alu.not_equal, fill=1.0,
                base=-127, pattern=[[1, 128]], channel_multiplier=1,
            )
            x_sb = constp.tile([128, T // 128, BL], i32, tag="x_sb")
            nc.sync.dma_start(
                out=x_sb[:, :, :],
                in_=x_t[:, :].rearrange("(j p) b -> p j b", p=128),
            )
            bias = {}
            nbias = {}
            for nm in ("b0f", "b0b", "b1f", "b1b"):
                bs = constp.tile([128, NK1], f32, tag=f"bias_{nm}")
                nc.sync.dma_start(
                    out=bs[:, :],
                    in_=b_t[nm][:].rearrange("(c p) -> p c", p=128),
                )
                nb = constp.tile([128, NK1], f32, tag=f"nbias_{nm}")
                nc.scalar.mul(nb[:, :], bs[:, :], -1.0)
                bias[nm] = bs
                nbias[nm] = nb
            wh_sb = constp.tile([128, NK1, C], f32, tag="wh")
            nc.sync.dma_start(
                out=wh_sb[:, :, :],
                in_=wh_t[:, :].rearrange("(c p) n -> p c n", p=128),
            )
            bh_sb = constp.tile([128, 1], f32, tag="bh")
            nc.sync.dma_start(out=bh_sb[:C, :1], in_=bh_t[:, None])
            z_all = constp.tile([128, NK1, BL], f32, tag="z_all")

            # W1f prefetch (lives through phase E into phase L1-forward)
            w1f_sb = w1fp.tile([128, NK1, 4 * H], DTS, tag="w1f")
            # bf16 needs the SWDGE cast path; fp32/f32r loads are cast-free
            dma_w = (nc.gpsimd.dma_start if mm_dtype == "bfloat16"
                     else nc.sync.dma_start)
            dma_w(
                out=w1f_sb[:, :, :],
                in_=dview(w_t["W1f"][:, :].rearrange("(c p) m -> p c m",
                                                     p=128)),
            )

            def sru_block(i, ps, bs, nbs, tmpp, dst, scratch_tag=None):
                """Consume gate PSUM tiles ps=[xt, fz, rz, hw] for one
                128-feature tile; write SRU output to dst (an AP) or, if
                scratch_tag, to a scratch tile returned to the caller."""
                f_tl = tmpp.tile([128, T], f32, tag="f_t")
                nc.scalar.activation(out=f_tl[:, :], in_=ps[1][:, :],
                                     func=Act.Sigmoid, bias=bs[:, i:i + 1])
                u_tl = tmpp.tile([128, T], f32, tag="u_t")
                nc.scalar.activation(out=u_tl[:, :], in_=ps[1][:, :],
                                     func=Act.Sigmoid, scale=-1.0,
                                     bias=nbs[:, i:i + 1])
                # u = sigmoid(-(fz+bf)) * xt = (1 - f) * xt
                nc.vector.tensor_tensor(out=u_tl[:, :], in0=u_tl[:, :],
                                        in1=ps[0][:, :], op=Alu.mult)
                c_tl = tmpp.tile([128, T], f32, tag="c_t")
                nc.vector.tensor_tensor_scan(
                    out=c_tl[:, :], data0=f_tl[:, :], data1=u_tl[:, :],
                    initial=0.0, op0=Alu.mult, op1=Alu.add)
                d_tl = tmpp.tile([128, T], f32, tag="d_t")
                nc.scalar.activation(out=d_tl[:, :], in_=c_tl[:, :],
                                     func=Act.Tanh)
                r_tl = tmpp.tile([128, T], f32, tag="r_t")
                nc.scalar.activation(out=r_tl[:, :], in_=ps[2][:, :],
                                     func=Act.Sigmoid, bias=bs[:, 4 + i:5 + i])
                # o = r * (tanh(c) - hw) + hw
                nc.vector.tensor_tensor(out=d_tl[:, :], in0=d_tl[:, :],
                                        in1=ps[3][:, :], op=Alu.subtract)
                nc.vector.tensor_tensor(out=r_tl[:, :], in0=r_tl[:, :],
                                        in1=d_tl[:, :], op=Alu.mult)
                if scratch_tag is not None:
                    o_tl = tmpp.tile([128, T], f32, tag=scratch_tag)
                    nc.vector.tensor_tensor(out=o_tl[:, :], in0=r_tl[:, :],
                                            in1=ps[3][:, :], op=Alu.add)
                    return o_tl
                nc.vector.tensor_tensor(out=dst, in0=r_tl[:, :],
                                        in1=ps[3][:, :], op=Alu.add)
                return None

            # ================= Phase E: embedding + layer 0 =================
            with tc.tile_pool(name="w0p", bufs=1) as w0p, \
                 tc.tile_pool(name="ep", bufs=2) as ep, \
                 tc.tile_pool(name="gp", bufs=4) as gp, \
                 tc.tile_pool(name="h0op", bufs=2) as h0op, \
                 tc.tile_pool(name="tmpE", bufs=2) as tmpE, \
                 tc.tile_pool(name="psE_tp", bufs=2, space="PSUM") as psE_tp, \
                 tc.tile_pool(name="psE_u", bufs=6, space="PSUM") as psE_u:

                w0_sb = {}
                for nm in ("W0f", "W0b"):
                    ws = w0p.tile([128, 3, 4 * H], DTS, tag=nm)
                    for kk, (_, ck) in enumerate(KCH0):
                        dma_w(out=ws[:ck, kk, :],
                              in_=dview(w_t[nm][128 * kk:128 * kk + ck, :]))
                    w0_sb[nm] = ws

                for b in range(BL):
                    eT = ep.tile([128, 3, T], DTS, tag="eT")
                    eTr = ep.tile([128, 3, T], DTS, tag="eTr")
                    for jj in range(T // 128):
                        g = gp.tile([128, D], f32, tag="g")
                        nc.gpsimd.indirect_dma_start(
                            out=g[:, :], out_offset=None,
                            in_=emb_t[:, :],
                            in_offset=IndirectOffsetOnAxis(
                                ap=x_sb[:, jj, b:b + 1], axis=0),
                        )
                        for cc, (_, cw) in enumerate(KCH0):
                            c0 = 128 * cc
                            tp = psE_tp.tile([128, 128], f32, tag="tp")
                            nc.tensor.transpose(out=tp[:cw, :],
                                                in_=g[:, c0:c0 + cw],
                                                identity=ident[:, :])
                            nc.vector.tensor_copy(
                                out=eT[:cw, cc, 128 * jj:128 * (jj + 1)],
                                in_=tp[:cw, :])
                            tpr = psE_tp.tile([128, 128], f32, tag="tp")
                            nc.tensor.transpose(out=tpr[:cw, :],
                                                in_=g[:, c0:c0 + cw],
                                                identity=antid[:, :])
                            nc.vector.tensor_copy(
                                out=eTr[:cw, cc, 128 * (3 - jj):128 * (4 - jj)],
                                in_=tpr[:cw, :])
                    h0sb = h0op.tile([128, NK1, T], DTS, tag="h0sb")
                    for di, (wnm, bnm, src) in enumerate(
                            (("W0f", "b0f", eT), ("W0b", "b0b", eTr))):
                        ws = w0_sb[wnm]
                        for i in range(4):
                            ps = []
                            for gi in range(4):
                                pt = psE_u.tile([128, T], f32, tag="upsE")
                                m0 = gi * H + i * 128
                                for kk, (_, ck) in enumerate(KCH0):
                                    nc.tensor.matmul(
                                        out=pt[:, :],
                                        lhsT=mmv(ws[:ck, kk, m0:m0 + 128]),
                                        rhs=mmv(src[:ck, kk, :]),
                                        start=(kk == 0),
                                        stop=(kk == len(KCH0) - 1))
                                ps.append(pt)
                            if di == 0:
                                dst = h0sb[:, i, :]
                            else:
                                dst = h0sb[:, 4 + i, ::-1]
                            sru_block(i, ps, bias[bnm], nbias[bnm], tmpE, dst)
                    nc.sync.dma_start(out=h0_dram[b], in_=h0sb[:, :, :])

            # ================= Phase L1: layer 1 + classifier ==============
            def l1_pass(wsb, bnm, reverse, h0ip, tmpp, psp, h0rp=None):
                for b in range(BL):
                    h0 = h0ip.tile([128, NK1, T], DTS, tag="h0i")
                    nc.sync.dma_start(out=h0[:, :, :], in_=h0_dram[b])
                    if reverse and not rev_rhs:
                        h0r = h0rp.tile([128, NK1, T], DTS, tag="h0r")
                        for kd in range(NK1):
                            nc.vector.tensor_copy(out=h0r[:, kd, :],
                                                  in_=h0[:, kd, ::-1])
                        src = h0r
                        rev_ap = False
                    else:
                        src = h0
                        rev_ap = reverse
                    for i in range(4):
                        ps = []
                        for gi in range(4):
                            pt = psp.tile([128, T], f32, tag="upsL")
                            m0 = gi * H + i * 128
                            for kk in range(NK1):
                                rhs = (src[:, kk, ::-1] if rev_ap
                                       else src[:, kk, :])
                                nc.tensor.matmul(
                                    out=pt[:, :],
                                    lhsT=mmv(wsb[:, kk, m0:m0 + 128]),
                                    rhs=mmv(rhs),
                                    start=(kk == 0), stop=(kk == NK1 - 1))
                            ps.append(pt)
                        o_tl = sru_block(i, ps, bias[bnm], nbias[bnm], tmpp,
                                         None, scratch_tag="o_t")
                        pm = tmpp.tile([128, 1], f32, tag="pm")
                        nc.vector.tensor_reduce(
                            out=pm[:, :1], in_=o_tl[:, :],
                            axis=mybir.AxisListType.X, op=Alu.max)
                        pm2 = tmpp.tile([128, 1], f32, tag="pm2")
                        nc.scalar.activation(out=pm2[:, :1], in_=pm[:, :1],
                                             func=Act.Tanh)
                        ci = (4 if reverse else 0) + i
                        nc.scalar.activation(out=z_all[:, ci, b:b + 1],
                                             in_=pm2[:, :1], func=Act.Tanh)

            with tc.tile_pool(name="h0ipf", bufs=2) as h0ipf, \
                 tc.tile_pool(name="tmpLf", bufs=2) as tmpLf, \
                 tc.tile_pool(name="psLf", bufs=6, space="PSUM") as psLf:
                l1_pass(w1f_sb, "b1f", False, h0ipf, tmpLf, psLf)

            with tc.tile_pool(name="w1bp", bufs=1) as w1bp, \
                 tc.tile_pool(name="h0ipb", bufs=2) as h0ipb, \
                 tc.tile_pool(name="h0rp", bufs=1) as h0rp, \
                 tc.tile_pool(name="tmpLb", bufs=2) as tmpLb, \
                 tc.tile_pool(name="psLb", bufs=6, space="PSUM") as psLb, \
                 tc.tile_pool(name="psCls", bufs=1, space="PSUM") as psCls:
                w1b_sb = w1bp.tile([128, NK1, 4 * H], DTS, tag="w1b")
                dma_w(
                    out=w1b_sb[:, :, :],
                    in_=dview(w_t["W1b"][:, :].rearrange("(c p) m -> p c m",
                                                         p=128)),
                )
                l1_pass(w1b_sb, "b1b", True, h0ipb, tmpLb, psLb, h0rp=h0rp)

                # classifier: out[c, b] = sum_k Wh[k, c] z[k, b] + bh[c]
                ocls = psCls.tile([C, BL], f32, tag="cls")
                for kk in range(NK1):
                    nc.tensor.matmul(out=ocls[:, :],
                                     lhsT=wh_sb[:, kk, :],
                                     rhs=z_all[:, kk, :],
                                     start=(kk == 0), stop=(kk == NK1 - 1))
                ob = tmpLb.tile([128, BL], f32, tag="ob")
                nc.vector.tensor_tensor(
                    out=ob[:C, :], in0=ocls[:, :],
                    in1=bh_sb[:C, :1].to_broadcast([C, BL]), op=Alu.add)
                nc.sync.dma_start(out=out_t[:, :], in_=ob[:C, :])

    nc.compile()
    return nc


_cache = {}


def _program():
    if "nc" not in _cache:
        _cache["nc"] = build_program()
    return _cache["nc"]


def make_in_maps(inputs):
    x = np.asarray(inputs["x"]).astype(np.int32)
    rep = {}
    for nm in ("embed", "W0f", "b0f", "W0b", "b0b", "W1f", "b1f", "W1b",
               "b1b", "Wh", "bh"):
        rep[nm] = np.ascontiguousarray(np.asarray(inputs[nm]),
                                       dtype=np.float32)
    in_maps = []
    for i in range(NCORES):
        m = dict(rep)
        m["x"] = np.ascontiguousarray(x[:, i * BL:(i + 1) * BL])
        in_maps.append(m)
    return in_maps


def run(inputs, trace=False):
    from concourse.bass_utils import run_bass_kernel_spmd
    nc = _program()
    res = run_bass_kernel_spmd(nc, make_in_maps(inputs),
                               list(range(NCORES)), trace=trace)
    _cache["last"] = res
    out = np.concatenate(
        [res.results[i]["out"].T for i in range(NCORES)], axis=0)
    return out.astype(np.float32), res


def kernel(**inputs):
    out, _ = run(inputs, trace=False)
    return out
